# revision 2
# baseline (speedup 1.0000x reference)
"""Trainium2 Bass kernel v2 for nn_Block_39513699123558 (gnn_message_passing).

Two layers of (Chebyshev graph conv K=5 -> BatchNorm -> ReLU) on
x[B=2, F0=16, V=162, X=Y=Z=16].

v2 strategy (vs v1): swap matmul operand roles so NO per-row SBUF
layout-bridge DMAs are needed (v1 spent 9.5ms of its 10.6ms issuing
11k tiny DMAs on the Sync queue).

  - 8 cores, data-parallel over B x S-quarters: each core owns
    [Fin, V, S=1024] (b = core//4, q = core%4).
  - Layer 1, per s8-chunk c (8 s-cols):
      cheb: stationary = x-slice [v, (j f)=128] (j = s-col-in-chunk,
      f), moving = Tstk [v, (k u)=648] (k=1..4, T_k symmetric,
      host-precomputed) -> psum [(j f), (k u)] -> evac to SBUF bf16.
      proj: stationary = block-diag W1 [(g f)=64, (g o)=128] per k,
      moving = evac'd cheb cols [64, 162] per k (k=0 term from a
      host-supplied xT layout), row-paired halves (tile_position
      (0,0) / (64,0)) accumulate 5 k's into psum y[(g o), 162].
      Conv bias cancels in BN -> dropped.
  - y rows (g,o) -> slab [128, c, h, u(192-pitch)].  BN stats via
    bn_stats on slab slices, bn_aggr, count-weighted (E,S) AllReduce,
    fold g-groups, per-partition scale/shift + ReLU in-place.
  - Layer 2, per s4-chunk (c,h): stationary = h1T tile built by xbar
    DMA-transpose (slab cols 0:128 and 64:192 -> [u, (g o)]), moving
    = same Tstk; proj k=0 reads normalized slab directly, k=1..4 read
    the evac'd cheb output; y2 overwrites slab in place.
  - BN2 finalize, normalize+ReLU, single bf16 store; host casts f32.
"""

import os
import sys

sys.path.insert(0, "/opt/trn_rl_repo")

SKIP_CC = os.environ.get("K_SKIP_CC", "0") == "1"

import numpy as np
import ml_dtypes

from concourse import bass, bacc, mybir
from concourse import tile
from concourse.bass_utils import run_bass_kernel_spmd

BF16 = ml_dtypes.bfloat16
BF = mybir.dt.bfloat16
F32 = mybir.dt.float32

V = 162
VA = 128
VB = V - VA  # 34
F1, F2 = 16, 32
K = 5
S = 1024          # s-columns per core
NC8 = S // 8      # 128 s8-chunks (layer 1)
NT = 2 * NC8      # 256 s4-chunks (layer 2)
KU = 4 * V        # 648 cheb-stack columns (k=1..4)
PIT = 192         # slab per-(c,h) pitch (u 0..161 + 30 pad)
EPS = 1e-5
N_CORES = 8
NBLK = 4          # layer-2 c-blocks
TPB = NT // NBLK  # 64 s4-chunks per block
NSL = 256         # bn-stats slots per layer (one per s4-chunk)


def build_program():
    nc = bacc.Bacc("TRN2", target_bir_lowering=False)
    xsta = nc.declare_dram_parameter("xsta", [VA, NC8 * 128], BF, False)
    xstb = nc.declare_dram_parameter("xstb", [VB, NC8 * 128], BF, False)
    xtd = nc.declare_dram_parameter("xtd", [128, NC8 * V], BF, False)
    tstk = nc.declare_dram_parameter("tstk", [V, KU], BF, False)
    w1t = nc.declare_dram_parameter("w1t", [128, K * 128], BF, False)
    tbd = nc.declare_dram_parameter("tbd", [128, KU], BF, False)
    w2t = nc.declare_dram_parameter("w2t", [128, K * 128], BF, False)
    gb1 = nc.declare_dram_parameter("gb1", [128, 2], F32, False)
    gb2 = nc.declare_dram_parameter("gb2", [128, 2], F32, False)
    wrow = nc.declare_dram_parameter("wrow", [128, 1], F32, False)
    idn = nc.declare_dram_parameter("idn", [128, 128], BF, False)
    out = nc.declare_dram_parameter("out", [128, NC8 * 2 * V], BF,
                                    isOutput=True)

    with tile.TileContext(nc) as tc:
        with (
            tc.tile_pool(name="consts", bufs=1) as cpool,
            tc.tile_pool(name="slab", bufs=1) as slpool,
            tc.tile_pool(name="stats", bufs=1) as spool,
            tc.tile_pool(name="dram", bufs=1, space="DRAM") as dram,
        ):
            tA = cpool.tile([VA, KU], BF)
            tB = cpool.tile([VB, KU], BF)
            w1tt = cpool.tile([128, K * 128], BF)
            tBd = cpool.tile([128, KU], BF)
            w2tt = cpool.tile([128, K * 128], BF)
            gb1t = cpool.tile([128, 2], F32)
            gb2t = cpool.tile([128, 2], F32)
            wrt = cpool.tile([128, 1], F32)
            idt = cpool.tile([128, 128], BF)
            nc.sync.dma_start(tA[:], tstk[0:VA, :])
            nc.sync.dma_start(tB[:], tstk[VA:V, :])
            nc.sync.dma_start(w1tt[:], w1t[:])
            nc.sync.dma_start(tBd[:], tbd[:])
            nc.sync.dma_start(w2tt[:], w2t[:])
            nc.sync.dma_start(gb1t[:], gb1[:])
            nc.sync.dma_start(gb2t[:], gb2[:])
            nc.sync.dma_start(wrt[:], wrow[:])
            nc.sync.dma_start(idt[:], idn[:])

            slab = slpool.tile([128, NT * PIT], BF)
            sl3 = slab[:, :].rearrange("p (t u) -> p t u", t=NT, u=PIT)
            stscr1 = spool.tile([128, NSL * 8], F32)
            stscr2 = spool.tile([128, NSL * 8], F32)

            def stat2(stscr, t0, src3):
                # one HW group per call: 6 stats/partition, FD<=512
                for j in range(2):
                    nc.vector.bn_stats(
                        stscr[:, (t0 + j) * 8:(t0 + j) * 8 + 6],
                        src3[:, j, :])
            par1 = spool.tile([128, 2], F32)
            par2 = spool.tile([128, 2], F32)

            def cheb(lhsA, lhsB, tBs, m1ps, xsp, evac2):
                # cheb: psum [(j f)|(g o), (k u)] in two 324-col banks
                cp0 = m1ps.tile([128, 324], F32, tag="cp0")
                cp1 = m1ps.tile([128, 324], F32, tag="cp1")
                nc.tensor.matmul(cp0[:], lhsA, tA[:, 0:324],
                                 start=True, stop=False)
                nc.tensor.matmul(cp0[:], lhsB, tBs[:, 0:324],
                                 start=False, stop=True)
                nc.tensor.matmul(cp1[:], lhsA, tA[:, 324:648],
                                 start=True, stop=False)
                nc.tensor.matmul(cp1[:], lhsB, tBs[:, 324:648],
                                 start=False, stop=True)
                xsTp = xsp.tile([128, KU], BF, tag="xsTp")
                nc.vector.tensor_copy(xsTp[:, 0:324], cp0[:])
                if evac2 == "v":
                    nc.vector.tensor_copy(xsTp[:, 324:648], cp1[:])
                else:
                    nc.scalar.copy(xsTp[:, 324:648], cp1[:])
                return xsTp

            def proj1(xsTp, rhs0A, rhs0B, m2ps):
                yA = m2ps.tile([128, V], F32, tag="yA")
                yB = m2ps.tile([128, V], F32, tag="yB")
                for k in range(K):
                    st = dict(start=(k == 0), stop=(k == K - 1))
                    rA = rhs0A if k == 0 else \
                        xsTp[0:64, (k - 1) * V:k * V]
                    rB = rhs0B if k == 0 else \
                        xsTp[64:128, (k - 1) * V:k * V]
                    nc.tensor.matmul(
                        yA[:], w1tt[0:64, k * 128:(k + 1) * 128], rA,
                        tile_position=(0, 0), **st)
                    nc.tensor.matmul(
                        yB[:], w1tt[64:128, k * 128:(k + 1) * 128], rB,
                        tile_position=(64, 0), **st)
                return yA, yB

            def proj2(xsTp, rhs0, m2ps):
                y2 = m2ps.tile([128, V], F32, tag="y2")
                for k in range(K):
                    st = dict(start=(k == 0), stop=(k == K - 1))
                    r = rhs0 if k == 0 else \
                        xsTp[:, (k - 1) * V:k * V]
                    nc.tensor.matmul(
                        y2[:], w2tt[:, k * 128:(k + 1) * 128], r, **st)
                return y2

            # ---- layer 1 ----
            with (
                tc.tile_pool(name="xin", bufs=1) as xpool,
                tc.tile_pool(name="m1ps", bufs=2, space="PSUM") as m1ps,
                tc.tile_pool(name="m2ps", bufs=2, space="PSUM") as m2ps,
                tc.tile_pool(name="xs", bufs=3) as xsp,
                tc.tile_pool(name="xtp", bufs=2) as xtp,
            ):
                xstA = xpool.tile([VA, NC8 * 128], BF)
                xstB = xpool.tile([VB, NC8 * 128], BF)
                QW = NC8 * 32
                for q4 in range(4):
                    nc.sync.dma_start(xstA[:, q4 * QW:(q4 + 1) * QW],
                                      xsta[:, q4 * QW:(q4 + 1) * QW])
                    nc.sync.dma_start(xstB[:, q4 * QW:(q4 + 1) * QW],
                                      xstb[:, q4 * QW:(q4 + 1) * QW])
                CB = 16  # xTd block size (chunks)
                pend = None

                def flush1(p):
                    c, xsTp, xtdb, cc = p
                    yA, yB = proj1(
                        xsTp,
                        xtdb[0:64, cc * V:(cc + 1) * V],
                        xtdb[64:128, cc * V:(cc + 1) * V], m2ps)
                    if c % 2 == 0:
                        nc.vector.tensor_copy(sl3[:, 2 * c, 0:V], yA[:])
                        nc.scalar.copy(sl3[:, 2 * c + 1, 0:V], yB[:])
                    else:
                        nc.scalar.copy(sl3[:, 2 * c, 0:V], yA[:])
                        nc.vector.tensor_copy(sl3[:, 2 * c + 1, 0:V], yB[:])
                    stat2(stscr1, 2 * c, sl3[:, 2 * c:2 * c + 2, 0:V])

                for c in range(NC8):
                    if c % CB == 0:
                        xTdb = xtp.tile([128, CB * V], BF, tag="xTdb")
                        nc.sync.dma_start(
                            xTdb[:], xtd[:, c * V:(c + CB) * V])
                    xsTp = cheb(
                        xstA[:, c * 128:(c + 1) * 128],
                        xstB[:, c * 128:(c + 1) * 128],
                        tBd[0:34], m1ps, xsp, "s")
                    if pend is not None:
                        flush1(pend)
                    pend = (c, xsTp, xTdb, c % CB)
                flush1(pend)
            def bn_finalize(stscr, gbt, par, tag):
                sv = stscr[:, :].rearrange("p (n e) -> p n e", n=NSL, e=8)
                mv = spool.tile([128, 2], F32, tag=f"mv{tag}")
                nc.vector.bn_aggr(mv[:], sv[:, :, 0:6])
                es = spool.tile([128, 2], F32, tag=f"es{tag}")
                nc.vector.tensor_mul(es[:, 1:2], mv[:, 0:1], mv[:, 0:1])
                nc.vector.tensor_add(es[:, 1:2], es[:, 1:2], mv[:, 1:2])
                nc.vector.tensor_copy(es[:, 0:1], mv[:, 0:1])
                nc.vector.tensor_mul(es[:, 0:1], es[:, 0:1], wrt[:, 0:1])
                nc.vector.tensor_mul(es[:, 1:2], es[:, 1:2], wrt[:, 0:1])
                cin = dram.tile([128, 2], F32, tag=f"cin{tag}")
                cout = dram.tile([128, 2], F32, tag=f"cout{tag}")
                nc.gpsimd.dma_start(cin[:], es[:])
                if not SKIP_CC:
                    nc.gpsimd.collective_compute(
                        "AllReduce", mybir.AluOpType.add,
                        replica_groups=[list(range(N_CORES))],
                        ins=[cin[:].opt()], outs=[cout[:].opt()])
                else:
                    nc.gpsimd.dma_start(cout[:], cin[:])
                qs = spool.tile([32, 8], F32, tag=f"qs{tag}")
                nc.sync.dma_start(
                    qs[:].rearrange("o (g e) -> o g e", g=4, e=2),
                    cout[:].rearrange("(g o) e -> o g e", g=4, o=32))
                acc = spool.tile([32, 6], F32, tag=f"acc{tag}")
                nc.vector.tensor_add(acc[:, 0:2], qs[:, 0:2], qs[:, 2:4])
                nc.vector.tensor_add(acc[:, 2:4], qs[:, 4:6], qs[:, 6:8])
                nc.vector.tensor_add(acc[:, 0:2], acc[:, 0:2], acc[:, 2:4])
                # acc[:,0]=mean, acc[:,1]=E[y^2]
                nc.vector.tensor_mul(acc[:, 2:3], acc[:, 0:1], acc[:, 0:1])
                nc.vector.tensor_sub(acc[:, 1:2], acc[:, 1:2], acc[:, 2:3])
                nc.vector.tensor_scalar_add(acc[:, 1:2], acc[:, 1:2], EPS)
                nc.scalar.sqrt(acc[:, 2:3], acc[:, 1:2])
                nc.vector.reciprocal(acc[:, 3:4], acc[:, 2:3])
                nc.vector.tensor_mul(acc[:, 4:5], gbt[0:32, 0:1], acc[:, 3:4])
                nc.vector.tensor_mul(acc[:, 5:6], acc[:, 0:1], acc[:, 4:5])
                nc.vector.tensor_sub(acc[:, 5:6], gbt[0:32, 1:2], acc[:, 5:6])
                for g in range(4):
                    nc.sync.dma_start(par[32 * g:32 * g + 32, 0:1],
                                      acc[:, 4:5])
                    nc.sync.dma_start(par[32 * g:32 * g + 32, 1:2],
                                      acc[:, 5:6])

            bn_finalize(stscr1, gb1t, par1, "1")
            # normalize + ReLU layer-1 slab in place (4 slices,
            # real u-columns only -- pad columns stay untouched)
            for i in range(4):
                t0, t1 = i * (NT // 4), (i + 1) * (NT // 4)
                nc.scalar.activation(
                    sl3[:, t0:t1, 0:V], sl3[:, t0:t1, 0:V],
                    mybir.ActivationFunctionType.Relu,
                    bias=par1[:, 1:2], scale=par1[:, 0:1])

            # ---- layer 2 ----
            with (
                tc.tile_pool(name="h1t", bufs=2) as hpool,
                tc.tile_pool(name="hbs", bufs=3) as hbp,
                tc.tile_pool(name="m1ps2", bufs=2, space="PSUM") as m1ps,
                tc.tile_pool(name="m2ps2", bufs=2, space="PSUM") as m2ps,
                tc.tile_pool(name="tps", bufs=2, space="PSUM") as tps,
                tc.tile_pool(name="xs2", bufs=3) as xsp,
            ):
                pend = None

                def flush2(p):
                    t, xsTp = p
                    y2 = proj2(xsTp, sl3[:, t, 0:V], m2ps)
                    nc.vector.tensor_copy(sl3[:, t, 0:V], y2[:])
                    if t % 2 == 1:
                        stat2(stscr2, t - 1, sl3[:, t - 1:t + 1, 0:V])

                for blk in range(NBLK):
                    hA = hpool.tile([128, TPB * 128], BF, tag="hA")
                    hbs = hbp.tile([34, TPB * 128], BF, tag="hbs")
                    for i in range(TPB):
                        t = blk * TPB + i
                        nc.sync.dma_start(
                            hA[:, i * 128:(i + 1) * 128],
                            sl3[:, t, 0:128], transpose=True)
                    # B-part transposes in one dense PE burst per block
                    for i in range(TPB):
                        t = blk * TPB + i
                        tp = tps.tile([34, 128], BF, tag="tp")
                        nc.tensor.transpose(tp[:], sl3[:, t, 128:162],
                                            idt[:])
                        nc.scalar.copy(hbs[:, i * 128:(i + 1) * 128],
                                       tp[:])
                    for i in range(TPB):
                        t = blk * TPB + i
                        xsTp = cheb(
                            hA[:, i * 128:(i + 1) * 128],
                            hbs[0:34, i * 128:(i + 1) * 128],
                            tBd[0:34], m1ps, xsp, "v")
                        if pend is not None:
                            flush2(pend)
                        pend = (t, xsTp)
                flush2(pend)

            bn_finalize(stscr2, gb2t, par2, "2")
            # normalize + ReLU into contiguous staging, then store
            with tc.tile_pool(name="stg", bufs=2) as stg:
                for i in range(4):
                    t0, t1 = i * (NT // 4), (i + 1) * (NT // 4)
                    so = stg.tile([128, (NT // 4) * V], BF, tag="stg")
                    nc.scalar.activation(
                        so[:, :].rearrange("p (t u) -> p t u",
                                           t=t1 - t0, u=V),
                        sl3[:, t0:t1, 0:V],
                        mybir.ActivationFunctionType.Relu,
                        bias=par2[:, 1:2], scale=par2[:, 0:1])
                    nc.sync.dma_start(out[:, t0 * V:t1 * V], so[:])
    nc.compile()
    return nc


def _host_prep(x, lap, w1, w2, g1, be1, g2, be2):
    lap64 = np.asarray(lap).astype(np.float64)
    T = [np.eye(V), lap64]
    for _ in range(2, K):
        T.append(2.0 * lap64 @ T[-1] - T[-2])
    # tstk[v, (k-1)*V + u] = T_k[u, v]
    tstk = np.concatenate([T[k].T for k in range(1, K)], axis=1)
    w1a = np.asarray(w1).astype(np.float64)   # [K, 16, 32]
    w2a = np.asarray(w2).astype(np.float64)   # [K, 32, 32]
    w1tt = np.zeros((128, K, 128), np.float64)
    w2tt = np.zeros((128, K, 128), np.float64)
    for g in range(4):
        for k in range(K):
            w1tt[g * 16:(g + 1) * 16, k, g * 32:(g + 1) * 32] = w1a[k]
            w1tt[64 + g * 16:64 + (g + 1) * 16, k,
                 g * 32:(g + 1) * 32] = w1a[k]
            w2tt[g * 32:(g + 1) * 32, k, g * 32:(g + 1) * 32] = w2a[k]
    tbd = np.zeros((128, KU), np.float64)
    tbd[0:VB] = tstk[VA:V]
    tbd[64:64 + VB] = tstk[VA:V]
    gb1 = np.zeros((128, 2), np.float32)
    gb2 = np.zeros((128, 2), np.float32)
    for g in range(4):
        gb1[g * 32:(g + 1) * 32, 0] = np.asarray(g1)
        gb1[g * 32:(g + 1) * 32, 1] = np.asarray(be1)
        gb2[g * 32:(g + 1) * 32, 0] = np.asarray(g2)
        gb2[g * 32:(g + 1) * 32, 1] = np.asarray(be2)
    wrow = np.full((128, 1), (0.25 if SKIP_CC else 1.0 / 32.0), np.float32)
    common = {
        "tstk": tstk.astype(BF16),
        "w1t": w1tt.reshape(128, K * 128).astype(BF16),
        "tbd": tbd.astype(BF16),
        "w2t": w2tt.reshape(128, K * 128).astype(BF16),
        "gb1": gb1, "gb2": gb2, "wrow": wrow,
        "idn": np.eye(128, dtype=np.float64).astype(BF16),
    }
    in_maps = []
    xf = np.asarray(x).reshape(2, F1, V, 4096)
    for core in range(N_CORES):
        b, q = core // 4, core % 4
        xs = xf[b, :, :, q * S:(q + 1) * S]     # [16, 162, 1024]
        # xsta[v, c, j, f] = xs[f, v, 8c+j]
        xst = xs.reshape(F1, V, NC8, 8).transpose(1, 2, 3, 0)
        # xtd[(h,g,f), c, u] = xs[f, u, 8c+4h+g]
        xt = xs.reshape(F1, V, NC8, 2, 4).transpose(3, 4, 0, 2, 1)
        m = dict(common)
        m["xsta"] = np.ascontiguousarray(
            xst[0:VA]).reshape(VA, NC8 * 128).astype(BF16)
        m["xstb"] = np.ascontiguousarray(
            xst[VA:V]).reshape(VB, NC8 * 128).astype(BF16)
        m["xtd"] = np.ascontiguousarray(xt).reshape(
            128, NC8 * V).astype(BF16)
        in_maps.append(m)
    return in_maps


_CACHE = {}


def _run(in_maps, trace=False):
    if "nc" not in _CACHE:
        _CACHE["nc"] = build_program()
    return run_bass_kernel_spmd(
        _CACHE["nc"], in_maps, core_ids=list(range(N_CORES)), trace=trace)


def kernel(x, lap, w1, b1, g1, be1, w2, b2, g2, be2, _trace=False):
    # conv biases b1/b2 cancel exactly inside BatchNorm -> ignored
    in_maps = _host_prep(x, lap, w1, w2, g1, be1, g2, be2)
    res = _run(in_maps, trace=_trace)
    _CACHE["last_results"] = res
    full = np.empty((2, F2, V, 4096), np.float32)
    for core in range(N_CORES):
        b, q = core // 4, core % 4
        # y[(g,o), c, h, u] -> out[o, u, 8c+4h+g]
        y = res.results[core]["out"].astype(np.float32).reshape(
            4, F2, NC8, 2, V)
        full[b, :, :, q * S:(q + 1) * S] = y.transpose(
            1, 4, 2, 3, 0).reshape(F2, V, S)
    return full.reshape(2, F2, V, 16, 16, 16)


# revision 3
# speedup vs baseline: 1.0082x; 1.0082x over previous
"""Trainium2 Bass kernel v2 for nn_Block_39513699123558 (gnn_message_passing).

Two layers of (Chebyshev graph conv K=5 -> BatchNorm -> ReLU) on
x[B=2, F0=16, V=162, X=Y=Z=16].

v2 strategy (vs v1): swap matmul operand roles so NO per-row SBUF
layout-bridge DMAs are needed (v1 spent 9.5ms of its 10.6ms issuing
11k tiny DMAs on the Sync queue).

  - 8 cores, data-parallel over B x S-quarters: each core owns
    [Fin, V, S=1024] (b = core//4, q = core%4).
  - Layer 1, per s8-chunk c (8 s-cols):
      cheb: stationary = x-slice [v, (j f)=128] (j = s-col-in-chunk,
      f), moving = Tstk [v, (k u)=648] (k=1..4, T_k symmetric,
      host-precomputed) -> psum [(j f), (k u)] -> evac to SBUF bf16.
      proj: stationary = block-diag W1 [(g f)=64, (g o)=128] per k,
      moving = evac'd cheb cols [64, 162] per k (k=0 term from a
      host-supplied xT layout), row-paired halves (tile_position
      (0,0) / (64,0)) accumulate 5 k's into psum y[(g o), 162].
      Conv bias cancels in BN -> dropped.
  - y rows (g,o) -> slab [128, c, h, u(192-pitch)].  BN stats via
    bn_stats on slab slices, bn_aggr, count-weighted (E,S) AllReduce,
    fold g-groups, per-partition scale/shift + ReLU in-place.
  - Layer 2, per s4-chunk (c,h): stationary = h1T tile built by xbar
    DMA-transpose (slab cols 0:128 and 64:192 -> [u, (g o)]), moving
    = same Tstk; proj k=0 reads normalized slab directly, k=1..4 read
    the evac'd cheb output; y2 overwrites slab in place.
  - BN2 finalize, normalize+ReLU, single bf16 store; host casts f32.
"""

import os
import sys

sys.path.insert(0, "/opt/trn_rl_repo")

SKIP_CC = os.environ.get("K_SKIP_CC", "0") == "1"

import numpy as np
import ml_dtypes

from concourse import bass, bacc, mybir
from concourse import tile
from concourse.bass_utils import run_bass_kernel_spmd

BF16 = ml_dtypes.bfloat16
BF = mybir.dt.bfloat16
F32 = mybir.dt.float32

V = 162
VA = 128
VB = V - VA  # 34
F1, F2 = 16, 32
K = 5
S = 1024          # s-columns per core
NC8 = S // 8      # 128 s8-chunks (layer 1)
NT = 2 * NC8      # 256 s4-chunks (layer 2)
KU = 4 * V        # 648 cheb-stack columns (k=1..4)
PIT = 192         # slab per-(c,h) pitch (u 0..161 + 30 pad)
EPS = 1e-5
N_CORES = 8
NBLK = 4          # layer-2 c-blocks
TPB = NT // NBLK  # 64 s4-chunks per block
NSL = 256         # bn-stats slots per layer (one per s4-chunk)


def build_program():
    nc = bacc.Bacc("TRN2", target_bir_lowering=False)
    xsta = nc.declare_dram_parameter("xsta", [VA, NC8 * 128], BF, False)
    xstb = nc.declare_dram_parameter("xstb", [VB, NC8 * 128], BF, False)
    xtd = nc.declare_dram_parameter("xtd", [128, NC8 * V], BF, False)
    tstk = nc.declare_dram_parameter("tstk", [V, KU], BF, False)
    w1t = nc.declare_dram_parameter("w1t", [128, K * 128], BF, False)
    tbd = nc.declare_dram_parameter("tbd", [128, KU], BF, False)
    w2t = nc.declare_dram_parameter("w2t", [128, K * 128], BF, False)
    gb1 = nc.declare_dram_parameter("gb1", [128, 2], F32, False)
    gb2 = nc.declare_dram_parameter("gb2", [128, 2], F32, False)
    wrow = nc.declare_dram_parameter("wrow", [128, 1], F32, False)
    idn = nc.declare_dram_parameter("idn", [128, 128], BF, False)
    out = nc.declare_dram_parameter("out", [128, NC8 * 2 * V], BF,
                                    isOutput=True)

    with tile.TileContext(nc) as tc:
        with (
            tc.tile_pool(name="consts", bufs=1) as cpool,
            tc.tile_pool(name="slab", bufs=1) as slpool,
            tc.tile_pool(name="stats", bufs=1) as spool,
            tc.tile_pool(name="dram", bufs=1, space="DRAM") as dram,
        ):
            tA = cpool.tile([VA, KU], BF)
            tB = cpool.tile([VB, KU], BF)
            w1tt = cpool.tile([128, K * 128], BF)
            tBd = cpool.tile([128, KU], BF)
            w2tt = cpool.tile([128, K * 128], BF)
            gb1t = cpool.tile([128, 2], F32)
            gb2t = cpool.tile([128, 2], F32)
            wrt = cpool.tile([128, 1], F32)
            idt = cpool.tile([128, 128], BF)
            nc.sync.dma_start(tA[:], tstk[0:VA, :])
            nc.sync.dma_start(tB[:], tstk[VA:V, :])
            nc.sync.dma_start(w1tt[:], w1t[:])
            nc.sync.dma_start(tBd[:], tbd[:])
            nc.sync.dma_start(w2tt[:], w2t[:])
            nc.sync.dma_start(gb1t[:], gb1[:])
            nc.sync.dma_start(gb2t[:], gb2[:])
            nc.sync.dma_start(wrt[:], wrow[:])
            nc.sync.dma_start(idt[:], idn[:])

            slab = slpool.tile([128, NT * PIT], BF)
            sl3 = slab[:, :].rearrange("p (t u) -> p t u", t=NT, u=PIT)
            sl4 = slab[:, :].rearrange("p (c h u) -> p c h u",
                                       c=NC8, h=2, u=PIT)
            stscr1 = spool.tile([128, NSL * 8], F32)
            stscr2 = spool.tile([128, NSL * 8], F32)

            def stat2(stscr, t0, src3):
                # one HW group per call: 6 stats/partition, FD<=512
                for j in range(2):
                    nc.vector.bn_stats(
                        stscr[:, (t0 + j) * 8:(t0 + j) * 8 + 6],
                        src3[:, j, :])
            par1 = spool.tile([128, 2], F32)
            par2 = spool.tile([128, 2], F32)

            def cheb(lhsA, lhsB, tBs, m1ps, xsTpD, q, evac2):
                # cheb psum [(j f)|(g o), (k u)] in two 324-col banks;
                # evac into pair tile cols (k, q, u)
                cp0 = m1ps.tile([128, 324], F32, tag="cp0")
                cp1 = m1ps.tile([128, 324], F32, tag="cp1")
                nc.tensor.matmul(cp0[:], lhsA, tA[:, 0:324],
                                 start=True, stop=False)
                nc.tensor.matmul(cp0[:], lhsB, tBs[:, 0:324],
                                 start=False, stop=True)
                nc.tensor.matmul(cp1[:], lhsA, tA[:, 324:648],
                                 start=True, stop=False)
                nc.tensor.matmul(cp1[:], lhsB, tBs[:, 324:648],
                                 start=False, stop=True)
                v4 = xsTpD[:, :].rearrange("p (k q u) -> p k q u",
                                           k=4, q=2, u=V)
                nc.vector.tensor_copy(
                    v4[:, 0:2, q, :],
                    cp0[:].rearrange("p (k u) -> p k u", k=2, u=V))
                if evac2 == "v":
                    nc.vector.tensor_copy(
                        v4[:, 2:4, q, :],
                        cp1[:].rearrange("p (k u) -> p k u", k=2, u=V))
                else:
                    nc.scalar.copy(
                        v4[:, 2:4, q, :],
                        cp1[:].rearrange("p (k u) -> p k u", k=2, u=V))

            def proj1(xsTpD, r0A, r0B, r1A, r1B, m2ps):
                yA = m2ps.tile([128, 2 * V], F32, tag="yA")
                yB = m2ps.tile([128, 2 * V], F32, tag="yB")
                w0 = w1tt
                nc.tensor.matmul(yA[:, 0:V], w0[0:64, 0:128], r0A,
                                 tile_position=(0, 0),
                                 start=True, stop=False)
                nc.tensor.matmul(yB[:, 0:V], w0[64:128, 0:128], r0B,
                                 tile_position=(64, 0),
                                 start=True, stop=False)
                nc.tensor.matmul(yA[:, V:2 * V], w0[0:64, 0:128], r1A,
                                 tile_position=(0, 0),
                                 start=False, stop=False)
                nc.tensor.matmul(yB[:, V:2 * V], w0[64:128, 0:128], r1B,
                                 tile_position=(64, 0),
                                 start=False, stop=False)
                for k in range(1, K):
                    st = dict(start=False, stop=(k == K - 1))
                    nc.tensor.matmul(
                        yA[:], w0[0:64, k * 128:(k + 1) * 128],
                        xsTpD[0:64, (k - 1) * 324:k * 324],
                        tile_position=(0, 0), **st)
                    nc.tensor.matmul(
                        yB[:], w0[64:128, k * 128:(k + 1) * 128],
                        xsTpD[64:128, (k - 1) * 324:k * 324],
                        tile_position=(64, 0), **st)
                return yA, yB

            def proj2(xsTpD, t0, m2ps):
                y2 = m2ps.tile([128, 2 * V], F32, tag="y2")
                nc.tensor.matmul(y2[:, 0:V], w2tt[:, 0:128],
                                 sl3[:, t0, 0:V],
                                 start=True, stop=False)
                nc.tensor.matmul(y2[:, V:2 * V], w2tt[:, 0:128],
                                 sl3[:, t0 + 1, 0:V],
                                 start=False, stop=False)
                for k in range(1, K):
                    nc.tensor.matmul(
                        y2[:], w2tt[:, k * 128:(k + 1) * 128],
                        xsTpD[:, (k - 1) * 324:k * 324],
                        start=False, stop=(k == K - 1))
                return y2

            # ---- layer 1 ----
            with (
                tc.tile_pool(name="xin", bufs=1) as xpool,
                tc.tile_pool(name="m1ps", bufs=2, space="PSUM") as m1ps,
                tc.tile_pool(name="m2ps", bufs=2, space="PSUM") as m2ps,
                tc.tile_pool(name="xs", bufs=3) as xsp,
                tc.tile_pool(name="xtp", bufs=2) as xtp,
            ):
                xstA = xpool.tile([VA, NC8 * 128], BF)
                xstB = xpool.tile([VB, NC8 * 128], BF)
                QW = NC8 * 32
                for q4 in range(4):
                    nc.sync.dma_start(xstA[:, q4 * QW:(q4 + 1) * QW],
                                      xsta[:, q4 * QW:(q4 + 1) * QW])
                    nc.sync.dma_start(xstB[:, q4 * QW:(q4 + 1) * QW],
                                      xstb[:, q4 * QW:(q4 + 1) * QW])
                CB = 16  # xTd block size (chunks)
                pend = None

                def flush1(p):
                    c0, xsTpD, xtdb, cc = p
                    yA, yB = proj1(
                        xsTpD,
                        xtdb[0:64, cc * V:(cc + 1) * V],
                        xtdb[64:128, cc * V:(cc + 1) * V],
                        xtdb[0:64, (cc + 1) * V:(cc + 2) * V],
                        xtdb[64:128, (cc + 1) * V:(cc + 2) * V], m2ps)
                    nc.vector.tensor_copy(
                        sl4[:, c0:c0 + 2, 0, 0:V],
                        yA[:].rearrange("p (q u) -> p q u", q=2, u=V))
                    nc.scalar.copy(
                        sl4[:, c0:c0 + 2, 1, 0:V],
                        yB[:].rearrange("p (q u) -> p q u", q=2, u=V))
                    stat2(stscr1, 2 * c0, sl3[:, 2 * c0:2 * c0 + 2, 0:V])
                    stat2(stscr1, 2 * c0 + 2,
                          sl3[:, 2 * c0 + 2:2 * c0 + 4, 0:V])

                for c0 in range(0, NC8, 2):
                    if c0 % CB == 0:
                        xTdb = xtp.tile([128, CB * V], BF, tag="xTdb")
                        nc.sync.dma_start(
                            xTdb[:], xtd[:, c0 * V:(c0 + CB) * V])
                    xsTpD = xsp.tile([128, 4 * 2 * V], BF, tag="xsTpD")
                    for q in range(2):
                        cheb(xstA[:, (c0 + q) * 128:(c0 + q + 1) * 128],
                             xstB[:, (c0 + q) * 128:(c0 + q + 1) * 128],
                             tBd[0:34], m1ps, xsTpD, q, "s")
                    if pend is not None:
                        flush1(pend)
                    pend = (c0, xsTpD, xTdb, c0 % CB)
                flush1(pend)

            def bn_finalize(stscr, gbt, par, tag):
                sv = stscr[:, :].rearrange("p (n e) -> p n e", n=NSL, e=8)
                mv = spool.tile([128, 2], F32, tag=f"mv{tag}")
                nc.vector.bn_aggr(mv[:], sv[:, :, 0:6])
                es = spool.tile([128, 2], F32, tag=f"es{tag}")
                nc.vector.tensor_mul(es[:, 1:2], mv[:, 0:1], mv[:, 0:1])
                nc.vector.tensor_add(es[:, 1:2], es[:, 1:2], mv[:, 1:2])
                nc.vector.tensor_copy(es[:, 0:1], mv[:, 0:1])
                nc.vector.tensor_mul(es[:, 0:1], es[:, 0:1], wrt[:, 0:1])
                nc.vector.tensor_mul(es[:, 1:2], es[:, 1:2], wrt[:, 0:1])
                cin = dram.tile([128, 2], F32, tag=f"cin{tag}")
                cout = dram.tile([128, 2], F32, tag=f"cout{tag}")
                nc.gpsimd.dma_start(cin[:], es[:])
                if not SKIP_CC:
                    nc.gpsimd.collective_compute(
                        "AllReduce", mybir.AluOpType.add,
                        replica_groups=[list(range(N_CORES))],
                        ins=[cin[:].opt()], outs=[cout[:].opt()])
                else:
                    nc.gpsimd.dma_start(cout[:], cin[:])
                qs = spool.tile([32, 8], F32, tag=f"qs{tag}")
                nc.sync.dma_start(
                    qs[:].rearrange("o (g e) -> o g e", g=4, e=2),
                    cout[:].rearrange("(g o) e -> o g e", g=4, o=32))
                acc = spool.tile([32, 6], F32, tag=f"acc{tag}")
                nc.vector.tensor_add(acc[:, 0:2], qs[:, 0:2], qs[:, 2:4])
                nc.vector.tensor_add(acc[:, 2:4], qs[:, 4:6], qs[:, 6:8])
                nc.vector.tensor_add(acc[:, 0:2], acc[:, 0:2], acc[:, 2:4])
                # acc[:,0]=mean, acc[:,1]=E[y^2]
                nc.vector.tensor_mul(acc[:, 2:3], acc[:, 0:1], acc[:, 0:1])
                nc.vector.tensor_sub(acc[:, 1:2], acc[:, 1:2], acc[:, 2:3])
                nc.vector.tensor_scalar_add(acc[:, 1:2], acc[:, 1:2], EPS)
                nc.scalar.sqrt(acc[:, 2:3], acc[:, 1:2])
                nc.vector.reciprocal(acc[:, 3:4], acc[:, 2:3])
                nc.vector.tensor_mul(acc[:, 4:5], gbt[0:32, 0:1], acc[:, 3:4])
                nc.vector.tensor_mul(acc[:, 5:6], acc[:, 0:1], acc[:, 4:5])
                nc.vector.tensor_sub(acc[:, 5:6], gbt[0:32, 1:2], acc[:, 5:6])
                for g in range(4):
                    nc.sync.dma_start(par[32 * g:32 * g + 32, 0:1],
                                      acc[:, 4:5])
                    nc.sync.dma_start(par[32 * g:32 * g + 32, 1:2],
                                      acc[:, 5:6])

            bn_finalize(stscr1, gb1t, par1, "1")
            # normalize + ReLU layer-1 slab in place (4 slices,
            # real u-columns only -- pad columns stay untouched)
            for i in range(4):
                t0, t1 = i * (NT // 4), (i + 1) * (NT // 4)
                nc.scalar.activation(
                    sl3[:, t0:t1, 0:V], sl3[:, t0:t1, 0:V],
                    mybir.ActivationFunctionType.Relu,
                    bias=par1[:, 1:2], scale=par1[:, 0:1])

            # ---- layer 2 ----
            with (
                tc.tile_pool(name="h1t", bufs=2) as hpool,
                tc.tile_pool(name="hbs", bufs=3) as hbp,
                tc.tile_pool(name="m1ps2", bufs=2, space="PSUM") as m1ps,
                tc.tile_pool(name="m2ps2", bufs=2, space="PSUM") as m2ps,
                tc.tile_pool(name="tps", bufs=2, space="PSUM") as tps,
                tc.tile_pool(name="xs2", bufs=3) as xsp,
            ):
                pend = None

                def flush2(p):
                    t0, xsTpD = p
                    y2 = proj2(xsTpD, t0, m2ps)
                    nc.vector.tensor_copy(
                        sl3[:, t0:t0 + 2, 0:V],
                        y2[:].rearrange("p (q u) -> p q u", q=2, u=V))
                    stat2(stscr2, t0, sl3[:, t0:t0 + 2, 0:V])

                for blk in range(NBLK):
                    hA = hpool.tile([128, TPB * 128], BF, tag="hA")
                    hbs = hbp.tile([34, TPB * 128], BF, tag="hbs")
                    for i in range(TPB):
                        t = blk * TPB + i
                        nc.sync.dma_start(
                            hA[:, i * 128:(i + 1) * 128],
                            sl3[:, t, 0:128], transpose=True)
                    # B-part transposes in one dense PE burst per block
                    for i in range(TPB):
                        t = blk * TPB + i
                        tp = tps.tile([34, 128], BF, tag="tp")
                        nc.tensor.transpose(tp[:], sl3[:, t, 128:162],
                                            idt[:])
                        nc.scalar.copy(hbs[:, i * 128:(i + 1) * 128],
                                       tp[:])
                    for i0 in range(0, TPB, 2):
                        t0 = blk * TPB + i0
                        xsTpD = xsp.tile([128, 4 * 2 * V], BF,
                                         tag="xsTpD2")
                        for q in range(2):
                            i = i0 + q
                            cheb(hA[:, i * 128:(i + 1) * 128],
                                 hbs[0:34, i * 128:(i + 1) * 128],
                                 tBd[0:34], m1ps, xsTpD, q, "s")
                        if pend is not None:
                            flush2(pend)
                        pend = (t0, xsTpD)
                flush2(pend)

            bn_finalize(stscr2, gb2t, par2, "2")
            # normalize + ReLU into contiguous staging, then store
            with tc.tile_pool(name="stg", bufs=2) as stg:
                for i in range(4):
                    t0, t1 = i * (NT // 4), (i + 1) * (NT // 4)
                    so = stg.tile([128, (NT // 4) * V], BF, tag="stg")
                    nc.scalar.activation(
                        so[:, :].rearrange("p (t u) -> p t u",
                                           t=t1 - t0, u=V),
                        sl3[:, t0:t1, 0:V],
                        mybir.ActivationFunctionType.Relu,
                        bias=par2[:, 1:2], scale=par2[:, 0:1])
                    nc.sync.dma_start(out[:, t0 * V:t1 * V], so[:])
    nc.compile()
    return nc


def _host_prep(x, lap, w1, w2, g1, be1, g2, be2):
    lap64 = np.asarray(lap).astype(np.float64)
    T = [np.eye(V), lap64]
    for _ in range(2, K):
        T.append(2.0 * lap64 @ T[-1] - T[-2])
    # tstk[v, (k-1)*V + u] = T_k[u, v]
    tstk = np.concatenate([T[k].T for k in range(1, K)], axis=1)
    w1a = np.asarray(w1).astype(np.float64)   # [K, 16, 32]
    w2a = np.asarray(w2).astype(np.float64)   # [K, 32, 32]
    w1tt = np.zeros((128, K, 128), np.float64)
    w2tt = np.zeros((128, K, 128), np.float64)
    for g in range(4):
        for k in range(K):
            w1tt[g * 16:(g + 1) * 16, k, g * 32:(g + 1) * 32] = w1a[k]
            w1tt[64 + g * 16:64 + (g + 1) * 16, k,
                 g * 32:(g + 1) * 32] = w1a[k]
            w2tt[g * 32:(g + 1) * 32, k, g * 32:(g + 1) * 32] = w2a[k]
    tbd = np.zeros((128, KU), np.float64)
    tbd[0:VB] = tstk[VA:V]
    tbd[64:64 + VB] = tstk[VA:V]
    gb1 = np.zeros((128, 2), np.float32)
    gb2 = np.zeros((128, 2), np.float32)
    for g in range(4):
        gb1[g * 32:(g + 1) * 32, 0] = np.asarray(g1)
        gb1[g * 32:(g + 1) * 32, 1] = np.asarray(be1)
        gb2[g * 32:(g + 1) * 32, 0] = np.asarray(g2)
        gb2[g * 32:(g + 1) * 32, 1] = np.asarray(be2)
    wrow = np.full((128, 1), (0.25 if SKIP_CC else 1.0 / 32.0), np.float32)
    common = {
        "tstk": tstk.astype(BF16),
        "w1t": w1tt.reshape(128, K * 128).astype(BF16),
        "tbd": tbd.astype(BF16),
        "w2t": w2tt.reshape(128, K * 128).astype(BF16),
        "gb1": gb1, "gb2": gb2, "wrow": wrow,
        "idn": np.eye(128, dtype=np.float64).astype(BF16),
    }
    in_maps = []
    xf = np.asarray(x).reshape(2, F1, V, 4096)
    for core in range(N_CORES):
        b, q = core // 4, core % 4
        xs = xf[b, :, :, q * S:(q + 1) * S]     # [16, 162, 1024]
        # xsta[v, c, j, f] = xs[f, v, 8c+j]
        xst = xs.reshape(F1, V, NC8, 8).transpose(1, 2, 3, 0)
        # xtd[(h,g,f), c, u] = xs[f, u, 8c+4h+g]
        xt = xs.reshape(F1, V, NC8, 2, 4).transpose(3, 4, 0, 2, 1)
        m = dict(common)
        m["xsta"] = np.ascontiguousarray(
            xst[0:VA]).reshape(VA, NC8 * 128).astype(BF16)
        m["xstb"] = np.ascontiguousarray(
            xst[VA:V]).reshape(VB, NC8 * 128).astype(BF16)
        m["xtd"] = np.ascontiguousarray(xt).reshape(
            128, NC8 * V).astype(BF16)
        in_maps.append(m)
    return in_maps


_CACHE = {}


def _run(in_maps, trace=False):
    if "nc" not in _CACHE:
        _CACHE["nc"] = build_program()
    return run_bass_kernel_spmd(
        _CACHE["nc"], in_maps, core_ids=list(range(N_CORES)), trace=trace)


def kernel(x, lap, w1, b1, g1, be1, w2, b2, g2, be2, _trace=False):
    # conv biases b1/b2 cancel exactly inside BatchNorm -> ignored
    in_maps = _host_prep(x, lap, w1, w2, g1, be1, g2, be2)
    res = _run(in_maps, trace=_trace)
    _CACHE["last_results"] = res
    full = np.empty((2, F2, V, 4096), np.float32)
    for core in range(N_CORES):
        b, q = core // 4, core % 4
        # y[(g,o), c, h, u] -> out[o, u, 8c+4h+g]
        y = res.results[core]["out"].astype(np.float32).reshape(
            4, F2, NC8, 2, V)
        full[b, :, :, q * S:(q + 1) * S] = y.transpose(
            1, 4, 2, 3, 0).reshape(F2, V, S)
    return full.reshape(2, F2, V, 16, 16, 16)
